# revision 1
# baseline (speedup 1.0000x reference)
"""Trainium2 Bass kernel for the attention-based encoder.

Computation (per batch b):
    a      = P @ y[b]                                  # [D]
    logits = x[b] @ a                                  # [M]
    p_un   = exp(logits - max(logits)); Z = sum(p_un)
    W[t]   = p_un[t-1] + p_un[t] + p_un[t+1] + p_un[t+2]  (zero-padded), W[M-1] = 0
    enc[b] = (W @ x[b]) / (Q * Z)                      # [D]

which is algebraically identical to the reference (cumsum sliding window +
bilinear softmax attention), with the smoothing window folded onto the softmax
weights instead of the embeddings so x[b] is only needed in natural layout.

Sharding: data-parallel over batch, 4 batches per core on 8 cores. P is
replicated (passed pre-transposed so the contraction dim lands on SBUF
partitions without on-chip transposes).
"""

import numpy as np

import concourse.bass as bass
import concourse.mybir as mybir
from concourse.tile import TileContext
from concourse.bass_utils import run_bass_kernel_spmd
from concourse.masks import make_identity

# ---------------------------------------------------------------------------
# This container's walrus supports only ONE sync wait per instruction ("Too
# many sync wait commands" at codegen otherwise), while Tile freely attaches
# several.  Post-pass: hoist excess waits onto injected same-engine NoOps
# placed immediately before the over-subscribed instruction.
# ---------------------------------------------------------------------------

_MAX_WAITS = 1


def split_sync_waits(nc: bass.Bass) -> None:
    uid = 0
    for fn in nc.m.functions:
        for blk in fn.blocks:
            new_insts = []
            for inst in blk.instructions:
                si = inst.sync_info
                waits = list(si.on_wait) if si and si.on_wait else []
                if len(waits) > _MAX_WAITS:
                    for w in waits[:-_MAX_WAITS]:
                        uid += 1
                        ev = mybir.InstEventSemaphore(
                            name=f"{inst.name}_hw{uid}",
                            opcode="EventSemaphore",
                            ins=[],
                            outs=[],
                            sync_info=mybir.SyncInfo(on_wait=[w], on_update=[]),
                        )
                        ev.engine = inst.engine
                        new_insts.append(ev)
                    si.on_wait = waits[-_MAX_WAITS:]
                new_insts.append(inst)
            blk.instructions[:] = new_insts

# ---------------------------------------------------------------------------

B, M, D, CD = 32, 2048, 1024, 5120
Q = 2
NCORES = 8
BPC = B // NCORES          # batches per core
NT = M // 128              # m-tiles per batch
KT = CD // 128             # k-tiles of the P contraction
F32 = mybir.dt.float32
ALU = mybir.AluOpType
AFT = mybir.ActivationFunctionType


def build_nc(reps: int = 1, n_batches: int = BPC, do_phase_b: bool = True,
             stop_after: str = "full", skip: tuple = ()) -> bass.Bass:
    nc = bass.Bass()
    xs = nc.declare_dram_parameter("xs", [BPC, M, D], F32, isOutput=False)
    pt = nc.declare_dram_parameter("pt", [CD, D], F32, isOutput=False)
    ys = nc.declare_dram_parameter("ys", [128, KT, BPC], F32, isOutput=False)
    enc = nc.declare_dram_parameter("enc", [BPC, D], F32, isOutput=True)

    with TileContext(nc) as tc:
        with (
            tc.tile_pool(name="const", bufs=1) as const_pool,
            tc.tile_pool(name="ysp", bufs=1) as ys_pool,
            tc.tile_pool(name="ptp", bufs=2) as pt_pool,
            tc.tile_pool(name="xp", bufs=2) as x_pool,
            tc.tile_pool(name="arep", bufs=1) as arep_pool,
            tc.tile_pool(name="small", bufs=1) as small_pool,
            tc.tile_pool(name="tiny", bufs=2) as tiny_pool,
            tc.tile_pool(name="scr", bufs=2) as scr_pool,
            tc.tile_pool(name="ps", bufs=1, space="PSUM") as psum_pool,
            tc.tile_pool(name="pse", bufs=2, space="PSUM") as psum_e_pool,
        ):
            ones_sb = const_pool.tile([1, 128], F32)
            nc.vector.memset(ones_sb[:], 1.0)
            ones_col = const_pool.tile([128, 1], F32)
            nc.vector.memset(ones_col[:], 1.0)
            nshift = const_pool.tile([128, 1], F32)
            nc.vector.memset(nshift[:], -16.0)
            ys_sb = const_pool.tile([128, KT, BPC], F32)
            nc.sync.dma_start(out=ys_sb[:], in_=ys[:])

            # banded matrices for W = S4 @ p (4-tap sliding-window sum done
            # as tiny matmuls in partition space): S4[j, m] = 1 iff j-m in
            # {-1, 0, 1, 2}; corner matrices carry the inter-tile halo.
            s4 = const_pool.tile([128, 128], F32)
            nc.gpsimd.memset(s4[:], 0.0)
            for base in (1, 0, -1, -2):
                nc.gpsimd.affine_select(
                    out=s4[:], in_=s4[:], compare_op=ALU.not_equal, fill=1.0,
                    base=base, pattern=[[-1, 128]], channel_multiplier=1,
                )
            sprev = const_pool.tile([128, 128], F32)
            nc.gpsimd.memset(sprev[:], 0.0)
            nc.gpsimd.affine_select(
                out=sprev[:], in_=sprev[:], compare_op=ALU.not_equal, fill=1.0,
                base=-127, pattern=[[-1, 128]], channel_multiplier=1,
            )
            snext = const_pool.tile([128, 128], F32)
            nc.gpsimd.memset(snext[:], 0.0)
            for base in (126, 127):
                nc.gpsimd.affine_select(
                    out=snext[:], in_=snext[:], compare_op=ALU.not_equal, fill=1.0,
                    base=base, pattern=[[-1, 128]], channel_multiplier=1,
                )

            # last-tile variant of s4 with column M-1 zeroed (W[M-1] = 0)
            s4last = const_pool.tile([128, 128], F32)
            nc.gpsimd.memset(s4last[:], 0.0)
            for base in (1, 0, -1, -2):
                nc.gpsimd.affine_select(
                    out=s4last[:], in_=s4last[:], compare_op=ALU.not_equal,
                    fill=1.0, base=base, pattern=[[-1, 128]],
                    channel_multiplier=1,
                )
            nc.gpsimd.affine_select(
                out=s4last[:], in_=s4last[:], compare_op=ALU.not_equal,
                fill=0.0, base=-127, pattern=[[1, 128]], channel_multiplier=0,
            )

            a_rep = [
                arep_pool.tile([128, D], F32, tag=f"a_rep{b}", name=f"a_rep{b}")
                for b in range(BPC)
            ]

            def body(_=None):
                if "phase_a" in skip:
                    [nc.vector.memset(ar[:], 0.001) for ar in a_rep]
                    aT_sb = small_pool.tile([BPC, D], F32, tag="aT")
                    nc.vector.memset(aT_sb[:], 0.001)
                    return body_b(aT_sb)

                # ---- Phase A: aT[b, d] = sum_k y[b, k] * PT[k, d] ----
                pa0 = psum_pool.tile([BPC, 512], F32, tag="pa0")
                pa1 = psum_pool.tile([BPC, 512], F32, tag="pa1")
                for c in range(KT // 2):
                    ptt = pt_pool.tile([128, 2, D], F32, tag="ptt")
                    nc.sync.dma_start(
                        out=ptt[:],
                        in_=pt[c * 256:(c + 1) * 256, :].rearrange(
                            "(u p) d -> p u d", p=128),
                    )
                    for u in range(2):
                        t = c * 2 + u
                        for dh, pa in enumerate((pa0, pa1)):
                            nc.tensor.matmul(
                                pa[:],
                                lhsT=ys_sb[:, t, :],
                                rhs=ptt[:, u, dh * 512:(dh + 1) * 512],
                                start=(t == 0),
                                stop=(t == KT - 1),
                            )
                aT_sb = small_pool.tile([BPC, D], F32, tag="aT")
                nc.scalar.copy(out=aT_sb[:, 0:512], in_=pa0[:])
                nc.scalar.copy(out=aT_sb[:, 512:1024], in_=pa1[:])

                # replicate a[b] across all 128 partitions (ones ⊗ a-row)
                for b in range(BPC):
                    a_row = small_pool.tile([1, D], F32, tag="a_row")
                    nc.sync.dma_start(out=a_row[:], in_=aT_sb[b:b + 1, :])
                    for dh in range(2):
                        pr = psum_pool.tile([128, 512], F32, tag="pr")
                        nc.tensor.matmul(
                            pr[:],
                            lhsT=ones_sb[:],
                            rhs=a_row[:, dh * 512:(dh + 1) * 512],
                            start=True,
                            stop=True,
                        )
                        nc.scalar.copy(
                            out=a_rep[b][:, dh * 512:(dh + 1) * 512], in_=pr[:]
                        )

                if not do_phase_b:
                    for b in range(BPC):
                        nc.sync.dma_start(out=enc[b, :], in_=aT_sb[b:b + 1, :])
                    return
                return body_b(aT_sb)

            def body_b(aT_sb):
                # ---- Phase B: per-batch attention ----
                for b in range(n_batches):
                    xb = x_pool.tile([128, NT, D], F32, tag="xb")
                    for t in range(NT // 4):
                        nc.sync.dma_start(
                            out=xb[:, 4 * t:4 * t + 4, :],
                            in_=xs[b, t * 512:(t + 1) * 512, :].rearrange(
                                "(u p) d -> p u d", p=128),
                        )

                    # logits[m] = x[m, :] . a  — DVE multiply, then ScalarE
                    # Copy-with-accumulate for the free-dim reduction (the
                    # fused TENSOR_TENSOR_REDUCE ISA op is rejected by this
                    # walrus build)
                    logits_sb = small_pool.tile([128, NT], F32, tag="logits")
                    if "logits" in skip:
                        nc.vector.memset(logits_sb[:], 0.01)
                    else:
                        for t in range(NT):
                            scratch = scr_pool.tile([128, D], F32, tag="scratch")
                            nc.vector.tensor_mul(
                                scratch[:], xb[:, t, :], a_rep[b][:]
                            )
                            nc.scalar.activation(
                                out=scratch[:],
                                in_=scratch[:],
                                func=AFT.Copy,
                                accum_out=logits_sb[:, t:t + 1],
                            )

                    if stop_after == "logits":
                        nc.sync.dma_start(out=enc[b, 0:NT], in_=logits_sb[0:1, :])
                        continue

                    if "softmax" in skip:
                        zsum = tiny_pool.tile([1, 1], F32, tag="zsum")
                        nc.vector.memset(zsum[:], 1.0)
                        w_pm = tiny_pool.tile([128, NT], F32, tag="w_pm")
                        nc.vector.memset(w_pm[:], 0.01)
                        do_tail(b, xb, w_pm, zsum)
                        continue

                    # softmax without the row gather: a FIXED shift replaces
                    # the max (it cancels exactly in enc = sum(W x)/(2Z)),
                    # so exp runs in [128, NT] partition space on ACT.
                    p_sb = tiny_pool.tile([128, NT], F32, tag="p_sb")
                    zcol = tiny_pool.tile([128, 1], F32, tag="zcol")
                    nc.scalar.activation(
                        out=p_sb[:],
                        in_=logits_sb[:],
                        func=AFT.Exp,
                        bias=nshift[:],
                        scale=1.0,
                        accum_out=zcol[:],
                    )

                    # Z = sum over partitions of zcol (ones-column matmul)
                    z_ps = psum_pool.tile([1, 1], F32, tag="pr")
                    nc.tensor.matmul(z_ps[:], lhsT=zcol[:], rhs=ones_col[:],
                                     start=True, stop=True)
                    zsum = tiny_pool.tile([1, 1], F32, tag="zsum")
                    nc.scalar.copy(out=zsum[:], in_=z_ps[:])

                    # W[m] = p[m-1]+p[m]+p[m+1]+p[m+2] via banded matmuls;
                    # inter-tile halo via corner matrices; W[M-1] = 0.
                    w_ps = psum_pool.tile([128, NT], F32, tag="w_ps")
                    for t in range(NT):
                        parts = [(s4last if t == NT - 1 else s4, t)]
                        if t > 0:
                            parts.append((sprev, t - 1))
                        if t < NT - 1:
                            parts.append((snext, t + 1))
                        for i, (mat, src) in enumerate(parts):
                            nc.tensor.matmul(
                                w_ps[:, t:t + 1], lhsT=mat[:],
                                rhs=p_sb[:, src:src + 1],
                                start=(i == 0), stop=(i == len(parts) - 1),
                            )
                    w_pm = tiny_pool.tile([128, NT], F32, tag="w_pm")
                    nc.scalar.copy(out=w_pm[:], in_=w_ps[:])

                    do_tail(b, xb, w_pm, zsum)

            def do_tail(b, xb, w_pm, zsum):
                # enc_un[d] = sum_m W[m] x[m, d]   (PE, W as 1-col weights)
                pe0 = psum_e_pool.tile([1, 512], F32, tag="pe0")
                pe1 = psum_e_pool.tile([1, 512], F32, tag="pe1")
                for t in range(NT):
                    for dh, pe in enumerate((pe0, pe1)):
                        nc.tensor.matmul(
                            pe[:],
                            lhsT=w_pm[:, t:t + 1],
                            rhs=xb[:, t, dh * 512:(dh + 1) * 512],
                            start=(t == 0),
                            stop=(t == NT - 1),
                        )

                if stop_after == "mm":
                    enc_sb0 = small_pool.tile([1, D], F32, tag="enc_sb")
                    nc.scalar.copy(out=enc_sb0[:, 0:512], in_=pe0[:])
                    nc.scalar.copy(out=enc_sb0[:, 512:1024], in_=pe1[:])
                    nc.sync.dma_start(out=enc[b, :], in_=enc_sb0[0:1, :])
                    return

                # enc[b] = enc_un / (Q * Z)
                z2 = small_pool.tile([1, 1], F32, tag="z2")
                nc.scalar.mul(out=z2[:], in_=zsum[:], mul=float(Q))
                rz = small_pool.tile([1, 1], F32, tag="rz")
                nc.vector.reciprocal(rz[:], z2[:])
                enc_sb = small_pool.tile([1, D], F32, tag="enc_sb")
                nc.scalar.activation(
                    out=enc_sb[:, 0:512], in_=pe0[:], func=AFT.Copy,
                    scale=rz[:],
                )
                nc.scalar.activation(
                    out=enc_sb[:, 512:1024], in_=pe1[:], func=AFT.Copy,
                    scale=rz[:],
                )
                nc.sync.dma_start(out=enc[b, :], in_=enc_sb[0:1, :])

            if reps == 1:
                body()
            else:
                with tc.For_i(0, reps, 1):
                    body()

    return nc


def _shard_inputs(embeds_x, embeds_y, P):
    """Build the 8 per-core input maps (host-side resharding)."""
    x = np.ascontiguousarray(np.asarray(embeds_x, dtype=np.float32))
    y = np.asarray(embeds_y, dtype=np.float32)[:, :, 0]          # [B, CD]
    pt = np.ascontiguousarray(np.asarray(P, dtype=np.float32).T)  # [CD, D]
    in_maps = []
    for c in range(NCORES):
        sl = slice(c * BPC, (c + 1) * BPC)
        ys_c = np.ascontiguousarray(
            y[sl].reshape(BPC, KT, 128).transpose(2, 1, 0)
        )  # [128, KT, BPC]
        in_maps.append({
            "xs": np.ascontiguousarray(x[sl]),
            "pt": pt,
            "ys": ys_c,
        })
    return in_maps


def kernel(embeds_x, embeds_y, P, M):
    assert int(M) == 2048
    nc = build_nc(reps=1)
    split_sync_waits(nc)  # HW-compile only; CoreSim rejects injected NoOps
    in_maps = _shard_inputs(embeds_x, embeds_y, P)
    res = run_bass_kernel_spmd(nc, in_maps, list(range(NCORES)))
    out = np.concatenate([res.results[c]["enc"] for c in range(NCORES)], axis=0)
    return out.astype(np.float32)



# revision 2
# speedup vs baseline: 1.3375x; 1.3375x over previous
"""Trainium2 Bass kernel for the attention-based encoder (v2, bf16 data path).

Computation (per batch b):
    a      = P @ y[b]                                  # [D]
    logits = x[b] @ a                                  # [M]
    p_un   = exp(logits - 16)                          # fixed shift (cancels)
    Z      = sum(p_un)
    W[t]   = p_un[t-1] + p_un[t] + p_un[t+1] + p_un[t+2]  (zero-padded), W[M-1] = 0
    enc[b] = (W @ x[b]) / (Q * Z)                      # [D]

v2 changes vs v1 (each driven by HW ablation of v1):
  * all HBM traffic in bf16 (x 16 MiB, P 10 MiB per core) - halves the DMA
    floor and runs every matmul at bf16 rate instead of multi-pass fp32
    (phase A was 161 us, mostly fp32 MM; enc MMs alone were ~110 us).
  * host pre-arranges x and P^T so every DMA is one large transfer with
    fully contiguous 32-80 KiB per partition.
  * W = 4-tap window of p computed with 3 banded matmuls of N=16 per batch
    (v1: 48 per batch of N=1; the block measured ~112 us).
  * a replicated across partitions with a selector matmul (v1 used a
    per-batch SBUF->SBUF DMA round trip).
  * logits dot-products: fused DVE affine_mul_reduce per m-tile when
    available ("amr"), else DVE mul + ACT accumulate, optionally split
    between ACT-accum and DVE tensor_reduce tiles to balance engines.

Sharding: data-parallel over batch, 4 batches per core on 8 cores.
"""

import numpy as np
import ml_dtypes

import concourse.bass as bass
import concourse.mybir as mybir
from concourse.tile import TileContext
from concourse.bass_utils import run_bass_kernel_spmd

# ---------------------------------------------------------------------------
# This container's walrus supports only ONE sync wait per instruction ("Too
# many sync wait commands" at codegen otherwise), while Tile freely attaches
# several.  Post-pass: hoist excess waits onto injected same-engine NoOps
# placed immediately before the over-subscribed instruction.
# ---------------------------------------------------------------------------

_MAX_WAITS = 1


def split_sync_waits(nc: bass.Bass) -> None:
    uid = 0
    for fn in nc.m.functions:
        for blk in fn.blocks:
            new_insts = []
            for inst in blk.instructions:
                si = inst.sync_info
                waits = list(si.on_wait) if si and si.on_wait else []
                if len(waits) > _MAX_WAITS:
                    for w in waits[:-_MAX_WAITS]:
                        uid += 1
                        ev = mybir.InstEventSemaphore(
                            name=f"{inst.name}_hw{uid}",
                            opcode="EventSemaphore",
                            ins=[],
                            outs=[],
                            sync_info=mybir.SyncInfo(on_wait=[w], on_update=[]),
                        )
                        ev.engine = inst.engine
                        new_insts.append(ev)
                    si.on_wait = waits[-_MAX_WAITS:]
                new_insts.append(inst)
            blk.instructions[:] = new_insts

# ---------------------------------------------------------------------------

B, M, D, CD = 32, 2048, 1024, 5120
Q = 2
NCORES = 8
BPC = B // NCORES          # batches per core
NT = M // 128              # m-tiles per batch
KT = CD // 128             # k-tiles of the P contraction
KCH = 8                    # k-tiles per P DMA chunk
NCH = KT // KCH
PAD = 1088                 # bf16 elems of per-partition-row padding: breaks
                           # the power-of-2 HBM stride aliasing (187 -> 315 GB/s)
HNT = NT // 2              # m-tiles per xb half-load
F32 = mybir.dt.float32
BF16 = mybir.dt.bfloat16
ALU = mybir.AluOpType
AFT = mybir.ActivationFunctionType


def build_nc(reps: int = 1, n_batches: int = BPC, do_phase_b: bool = True,
             stop_after: str = "full", skip: tuple = (),
             logits_mode: str = "split", xhalves: bool = False,
             n_dve: int = 16, n_gps: int = 0) -> bass.Bass:
    nc = bass.Bass()
    xs = nc.declare_dram_parameter("xs", [128, BPC, NT * D + PAD], BF16, isOutput=False)
    pt = nc.declare_dram_parameter("pt", [128, KT * D + PAD], BF16, isOutput=False)
    ys = nc.declare_dram_parameter("ys", [128, KT, BPC], BF16, isOutput=False)
    enc = nc.declare_dram_parameter("enc", [1, BPC * D], F32, isOutput=True)

    with TileContext(nc) as tc:
        with (
            tc.tile_pool(name="const", bufs=1) as const_pool,
            tc.tile_pool(name="ysp", bufs=1) as ys_pool,
            tc.tile_pool(name="ptp", bufs=3) as pt_pool,
            tc.tile_pool(name="xp", bufs=3) as x_pool,
            tc.tile_pool(name="arep", bufs=1) as arep_pool,
            tc.tile_pool(name="small", bufs=1) as small_pool,
            tc.tile_pool(name="tiny", bufs=2) as tiny_pool,
            tc.tile_pool(name="scr", bufs=3) as scr_pool,
            tc.tile_pool(name="ps", bufs=1, space="PSUM") as psum_pool,
            tc.tile_pool(name="pse", bufs=2, space="PSUM") as psum_e_pool,
        ):
            ones_col = const_pool.tile([128, 1], F32)
            nc.vector.memset(ones_col[:], 1.0)
            nshift = const_pool.tile([128, 1], F32)
            nc.vector.memset(nshift[:], -16.0)
            ys_sb = const_pool.tile([128, KT, BPC], BF16)
            nc.sync.dma_start(out=ys_sb[:], in_=ys[:])

            # banded matrices for the 4-tap sliding-window sum W = S4 @ p:
            # s4[c, f] = 1 iff f - c in {-2, -1, 0, 1}; corner matrices
            # carry the +-2-element inter-tile halo.
            s4 = const_pool.tile([128, 128], F32)
            nc.gpsimd.memset(s4[:], 0.0)
            for base in (1, 0, -1, -2):
                nc.gpsimd.affine_select(
                    out=s4[:], in_=s4[:], compare_op=ALU.not_equal, fill=1.0,
                    base=base, pattern=[[-1, 128]], channel_multiplier=1,
                )
            sprev = const_pool.tile([128, 128], F32)
            nc.gpsimd.memset(sprev[:], 0.0)
            nc.gpsimd.affine_select(
                out=sprev[:], in_=sprev[:], compare_op=ALU.not_equal, fill=1.0,
                base=-127, pattern=[[-1, 128]], channel_multiplier=1,
            )
            snext = const_pool.tile([128, 128], F32)
            nc.gpsimd.memset(snext[:], 0.0)
            for base in (126, 127):
                nc.gpsimd.affine_select(
                    out=snext[:], in_=snext[:], compare_op=ALU.not_equal, fill=1.0,
                    base=base, pattern=[[-1, 128]], channel_multiplier=1,
                )

            # last-tile variant of s4 with column M-1 zeroed (W[M-1] = 0)
            s4last = const_pool.tile([128, 128], F32)
            nc.gpsimd.memset(s4last[:], 0.0)
            for base in (1, 0, -1, -2):
                nc.gpsimd.affine_select(
                    out=s4last[:], in_=s4last[:], compare_op=ALU.not_equal,
                    fill=1.0, base=base, pattern=[[-1, 128]],
                    channel_multiplier=1,
                )
            nc.gpsimd.affine_select(
                out=s4last[:], in_=s4last[:], compare_op=ALU.not_equal,
                fill=0.0, base=-127, pattern=[[1, 128]], channel_multiplier=0,
            )

            # selector rows: selb[b][k, m] = 1 iff k == b (4 partitions);
            # a_rep[b] = selb[b].T @ aT broadcasts batch b's row of aT to
            # all 128 partitions without an SBUF round-trip.
            selb = []
            for b in range(BPC):
                sb = const_pool.tile([BPC, 128], BF16, name=f"selb{b}")
                nc.gpsimd.memset(sb[:], 0.0)
                nc.gpsimd.affine_select(
                    out=sb[:], in_=sb[:], compare_op=ALU.not_equal, fill=1.0,
                    base=-b, pattern=[[0, 128]], channel_multiplier=1,
                )
                selb.append(sb)

            a_rep = [
                arep_pool.tile([128, D], BF16, tag=f"a_rep{b}", name=f"a_rep{b}")
                for b in range(BPC)
            ]

            def body(_=None):
                if "phase_a" in skip:
                    [nc.vector.memset(ar[:], 0.001) for ar in a_rep]
                    return body_b()

                # ---- Phase A: aT[b, d] = sum_k y[b, k] * PT[k, d] ----
                pa0 = psum_pool.tile([BPC, 512], F32, tag="pa0")
                pa1 = psum_pool.tile([BPC, 512], F32, tag="pa1")
                for c in range(NCH):
                    ptc = pt_pool.tile([128, KCH * D], BF16, tag="ptc")
                    nc.sync.dma_start(
                        out=ptc[:], in_=pt[:, c * KCH * D:(c + 1) * KCH * D]
                    )
                    for u in range(KCH):
                        t = c * KCH + u
                        nc.tensor.matmul(
                            pa0[:], lhsT=ys_sb[:, t, :],
                            rhs=ptc[:, u * D:u * D + 512],
                            start=(t == 0), stop=(t == KT - 1),
                        )
                        nc.tensor.matmul(
                            pa1[:], lhsT=ys_sb[:, t, :],
                            rhs=ptc[:, u * D + 512:(u + 1) * D],
                            start=(t == 0), stop=(t == KT - 1),
                        )
                aT_sb = small_pool.tile([BPC, D], BF16, tag="aT")
                nc.vector.tensor_copy(aT_sb[:, 0:512], pa0[:])
                nc.vector.tensor_copy(aT_sb[:, 512:1024], pa1[:])

                # replicate a[b] across all 128 partitions (selector matmul)
                for b in range(BPC):
                    for dh in range(2):
                        pr = psum_pool.tile([128, 512], F32, tag="pr")
                        nc.tensor.matmul(
                            pr[:], lhsT=selb[b][:],
                            rhs=aT_sb[:, dh * 512:(dh + 1) * 512],
                            start=True, stop=True,
                        )
                        nc.vector.tensor_copy(
                            a_rep[b][:, dh * 512:(dh + 1) * 512], pr[:]
                        )

                if not do_phase_b:
                    for b in range(BPC):
                        nc.gpsimd.dma_start(
                            out=enc[0, b * D:b * D + 512],
                            in_=a_rep[b][0:1, 0:512],
                        )
                    return
                return body_b()

            def body_b():
                # ---- Phase B: per-batch attention ----
                # n_dve: reduce-halves handed to DVE tensor_reduce instead of
                # ACT accumulate; n_gps: muls offloaded to GpSimd.
                for b in range(n_batches):
                    if xhalves:
                        # two half-batch loads so compute starts ~6.5us earlier
                        xh = []
                        for h in range(2):
                            xt_ = x_pool.tile([128, HNT * D], BF16, tag=f"xh{h}")
                            nc.sync.dma_start(
                                out=xt_[:],
                                in_=xs[:, b, h * HNT * D:(h + 1) * HNT * D],
                            )
                            xh.append(xt_)
                    else:
                        xb = x_pool.tile([128, NT * D], BF16, tag="xh0")
                        nc.sync.dma_start(out=xb[:], in_=xs[:, b, 0:NT * D])
                        xh = [xb[:, 0:HNT * D], xb[:, HNT * D:NT * D]]

                    # logits[m] = x[m, :] . a  - DVE mul per m-tile, then the
                    # free-dim reduce as two 512-halves on ACT (420ns each vs
                    # 1439ns for a 1024 accum / 1266ns for a DVE reduce).
                    logits_a = tiny_pool.tile([128, NT], F32, tag="logits_a")
                    logits_b = tiny_pool.tile([128, NT], F32, tag="logits_b")
                    if "logits" in skip:
                        nc.vector.memset(logits_a[:], 0.005)
                        nc.vector.memset(logits_b[:], 0.005)
                    else:
                        for t in range(NT):
                            xt = xh[t // HNT][:, (t % HNT) * D:(t % HNT + 1) * D]
                            scratch = scr_pool.tile([128, D], BF16, tag="scratch")
                            mul_eng = nc.gpsimd if t < n_gps else nc.vector
                            mul_eng.tensor_mul(scratch[:], xt, a_rep[b][:])
                            if t < n_dve - NT:
                                nc.vector.tensor_reduce(
                                    out=logits_a[:, t:t + 1],
                                    in_=scratch[:, 0:512],
                                    axis=mybir.AxisListType.X, op=ALU.add,
                                )
                            else:
                                nc.scalar.activation(
                                    out=scratch[:, 0:512], in_=scratch[:, 0:512],
                                    func=AFT.Copy, accum_out=logits_a[:, t:t + 1],
                                )
                            if t < n_dve:
                                nc.vector.tensor_reduce(
                                    out=logits_b[:, t:t + 1],
                                    in_=scratch[:, 512:1024],
                                    axis=mybir.AxisListType.X, op=ALU.add,
                                )
                            else:
                                nc.scalar.activation(
                                    out=scratch[:, 512:1024],
                                    in_=scratch[:, 512:1024],
                                    func=AFT.Copy,
                                    accum_out=logits_b[:, t:t + 1],
                                )
                    nc.vector.tensor_add(logits_a[:], logits_a[:], logits_b[:])

                    if stop_after == "logits":
                        nc.sync.dma_start(out=enc[0, b * D:b * D + NT], in_=logits_a[0:1, :])
                        continue

                    if "softmax" in skip:
                        zsum = tiny_pool.tile([1, 1], F32, tag="zsum")
                        nc.vector.memset(zsum[:], 1.0)
                        w_pm = tiny_pool.tile([128, NT], BF16, tag="w_pm")
                        nc.vector.memset(w_pm[:], 0.01)
                        do_tail(b, xh, w_pm, zsum)
                        continue

                    # softmax without the row max: fixed shift (cancels in
                    # enc = sum(W x)/(Q Z)); exp on ACT in [128, NT] space.
                    p_pad = tiny_pool.tile([128, NT + 2], F32, tag="p_pad")
                    zcol = tiny_pool.tile([128, 1], F32, tag="zcol")
                    nc.vector.memset(p_pad[:, 0:1], 0.0)
                    nc.vector.memset(p_pad[:, NT + 1:NT + 2], 0.0)
                    nc.scalar.activation(
                        out=p_pad[:, 1:NT + 1],
                        in_=logits_a[:],
                        func=AFT.Exp,
                        bias=nshift[:],
                        scale=1.0,
                    )
                    nc.vector.tensor_reduce(
                        out=zcol[:], in_=p_pad[:, 1:NT + 1],
                        axis=mybir.AxisListType.X, op=ALU.add,
                    )

                    # Z = sum over partitions of zcol (ones-column matmul)
                    z_ps = psum_pool.tile([1, 1], F32, tag="pr")
                    nc.tensor.matmul(z_ps[:], lhsT=zcol[:], rhs=ones_col[:],
                                     start=True, stop=True)
                    zsum = tiny_pool.tile([1, 1], F32, tag="zsum")
                    nc.scalar.copy(out=zsum[:], in_=z_ps[:])

                    # W = 4-tap window of p: banded matmuls over tiles
                    # 0..14 at once (halo via shifted rhs columns of p_pad);
                    # the last tile separately with s4last (W[M-1] = 0).
                    w_ps = psum_pool.tile([128, NT], F32, tag="w_ps")
                    nc.tensor.matmul(w_ps[:, 0:NT - 1], lhsT=s4[:],
                                     rhs=p_pad[:, 1:NT], start=True, stop=False)
                    nc.tensor.matmul(w_ps[:, 0:NT - 1], lhsT=sprev[:],
                                     rhs=p_pad[:, 0:NT - 1], start=False,
                                     stop=False)
                    nc.tensor.matmul(w_ps[:, 0:NT - 1], lhsT=snext[:],
                                     rhs=p_pad[:, 2:NT + 1], start=False,
                                     stop=True)
                    nc.tensor.matmul(w_ps[:, NT - 1:NT], lhsT=s4last[:],
                                     rhs=p_pad[:, NT:NT + 1], start=True,
                                     stop=False)
                    nc.tensor.matmul(w_ps[:, NT - 1:NT], lhsT=sprev[:],
                                     rhs=p_pad[:, NT - 1:NT], start=False,
                                     stop=True)
                    w_pm = tiny_pool.tile([128, NT], BF16, tag="w_pm")
                    nc.scalar.copy(out=w_pm[:], in_=w_ps[:])

                    do_tail(b, xh, w_pm, zsum)

            def do_tail(b, xh, w_pm, zsum):
                # enc_un[d] = sum_m W[m] x[m, d]   (PE, W cols as weights)
                pe0 = psum_e_pool.tile([1, 512], F32, tag="pe0")
                pe1 = psum_e_pool.tile([1, 512], F32, tag="pe1")
                for t in range(NT):
                    xt = xh[t // HNT][:, (t % HNT) * D:(t % HNT + 1) * D]
                    for dh, pe in enumerate((pe0, pe1)):
                        nc.tensor.matmul(
                            pe[:],
                            lhsT=w_pm[:, t:t + 1],
                            rhs=xt[:, dh * 512:(dh + 1) * 512],
                            start=(t == 0),
                            stop=(t == NT - 1),
                        )

                enc_sb = small_pool.tile([1, BPC * D], F32, tag="enc_sb")
                if stop_after == "mm":
                    nc.scalar.copy(out=enc_sb[:, b * D:b * D + 512], in_=pe0[:])
                    nc.scalar.copy(out=enc_sb[:, b * D + 512:(b + 1) * D],
                                   in_=pe1[:])
                    if b == n_batches - 1:
                        nc.sync.dma_start(out=enc[:], in_=enc_sb[0:1, :])
                    return

                # enc[b] = enc_un / (Q * Z)
                z2 = small_pool.tile([1, 1], F32, tag="z2")
                nc.scalar.mul(out=z2[:], in_=zsum[:], mul=float(Q))
                rz = small_pool.tile([1, 1], F32, tag="rz")
                nc.vector.reciprocal(rz[:], z2[:])
                nc.scalar.activation(
                    out=enc_sb[:, b * D:b * D + 512], in_=pe0[:], func=AFT.Copy,
                    scale=rz[:],
                )
                nc.scalar.activation(
                    out=enc_sb[:, b * D + 512:(b + 1) * D], in_=pe1[:],
                    func=AFT.Copy, scale=rz[:],
                )
                if b == n_batches - 1:
                    nc.sync.dma_start(out=enc[:], in_=enc_sb[0:1, :])

            if reps == 1:
                body()
            else:
                with tc.For_i(0, reps, 1):
                    body()

    return nc


def _shard_inputs(embeds_x, embeds_y, P):
    """Build the 8 per-core input maps (host-side layout + bf16 cast)."""
    bf = ml_dtypes.bfloat16
    x = np.asarray(embeds_x, dtype=np.float32)
    y = np.asarray(embeds_y, dtype=np.float32)[:, :, 0]          # [B, CD]
    # pt[p, k*D + d] = P[d, k*128 + p]
    ptr = np.zeros((128, KT * D + PAD), dtype=bf)
    ptr[:, :KT * D] = P.T.reshape(KT, 128, D).transpose(1, 0, 2).reshape(
        128, KT * D).astype(bf)
    in_maps = []
    for c in range(NCORES):
        sl = slice(c * BPC, (c + 1) * BPC)
        # xs[p, b, t*D + d] = x[b, t*128 + p, d]
        xs_c = np.zeros((128, BPC, NT * D + PAD), dtype=bf)
        xs_c[:, :, :NT * D] = x[sl].reshape(BPC, NT, 128, D).transpose(
            2, 0, 1, 3).reshape(128, BPC, NT * D).astype(bf)
        ys_c = np.ascontiguousarray(
            y[sl].reshape(BPC, KT, 128).transpose(2, 1, 0)
        ).astype(bf)  # [128, KT, BPC]
        in_maps.append({"xs": xs_c, "pt": ptr, "ys": ys_c})
    return in_maps


def kernel(embeds_x, embeds_y, P, M):
    assert int(M) == 2048
    nc = build_nc(reps=1, xhalves=True)
    split_sync_waits(nc)  # HW-compile only; CoreSim rejects injected NoOps
    in_maps = _shard_inputs(embeds_x, embeds_y, P)
    res = run_bass_kernel_spmd(nc, in_maps, list(range(NCORES)))
    out = np.concatenate(
        [res.results[c]["enc"].reshape(BPC, D) for c in range(NCORES)], axis=0)
    return out.astype(np.float32)
